# revision 24
# baseline (speedup 1.0000x reference)
"""Multi-head graph attention (GAT) Trainium2 kernel, 8-core SPMD.

Problem: h[4096,256], adj[4096,4096] bool, w[4,256,64], a_src/a_dst[4,64,1],
bias[64] -> out[4096,4,64]:
    h_prime = h @ w[k]                       per head
    s[i,j]  = src[i] + dst[j]                (rank-1!)
    scores  = leaky_relu(s, 0.2), masked by adj, softmax over j
    out     = attn @ h_prime + bias

Sharding: 8 cores = 2 head-groups x 4 row-blocks. Core c computes heads
[2*(c%2), 2*(c%2)+1] for output rows [1024*(c//2), 1024*(c//2)+1024).

Key algebra (all on-device, per head): any factor depending only on the
output row i cancels in the softmax, so the unnormalized weight can be
taken as
    P[j,i] = adj[i,j] * max(e^{0.8 dst_j}, e^{-0.8 src_i})
(equal to the true numerator divided by e^{0.2 src_i + 0.8 src_i}), which
is ONE fused DVE op per tile: scalar_tensor_tensor with op0=max against
the per-partition scalar e^{0.8 dst_j} and op1=mult against the adj mask.
The per-j factor e^{0.2 dst_j - 20} rides the stationary bmm operand
G = e^{0.2 dst - 20} * [h_prime | 1]; the ones-column makes the matmul's
last row the softmax denominator, and the epilogue divides it out.
"""

import sys

sys.path.insert(0, "/opt/trn_rl_repo")

import numpy as np
import ml_dtypes

N = 4096          # nodes
F = 256           # f_in
O = 64            # f_out
NHEAD = 4
NCORES = 8
NH = 2            # heads per core
NI = 1024         # output rows per core
NCJ = N // 128    # 32 j-chunks
NSEG = NI // 512  # 2 bmm segments of 512 (PSUM/matmul moving-dim limit)
NSUB = NI // 128  # 8 i-subtiles of 128
CB = 20.0         # shift inside e^{0.2 dst - CB} to keep bf16 range safe
PIPE = 2          # h_prime producer chunks in flight (kept short so the
                  # hT stream isn't demanded ahead of the adjT pacing)
NWARM = 10        # PE warm-up matmuls (beat the p-state/HAM ramp)
GLEAD = 2         # how many chunks ahead GPSIMD mask tiles are issued

_CACHE = {}


def _build():
    import concourse.bass as bass
    import concourse.bacc as bacc
    import concourse.mybir as mybir
    import concourse.tile as tile
    from concourse.bass import ts

    from concourse.masks import make_identity

    f32 = mybir.dt.float32
    bf16 = mybir.dt.bfloat16
    Alu = mybir.AluOpType
    Act = mybir.ActivationFunctionType

    nc = bacc.Bacc()
    hT_d = nc.declare_dram_parameter("hT", [F, N], bf16, isOutput=False)
    adjT_d = nc.declare_dram_parameter("adjT", [8 * 128, 4 * NI], bf16, isOutput=False)
    wr_d = nc.declare_dram_parameter("wr", [F, NH * O], bf16, isOutput=False)
    wTr_d = nc.declare_dram_parameter("wTr", [O, NH * F], bf16, isOutput=False)
    avec_d = nc.declare_dram_parameter("avec", [O, 2 * NH], bf16, isOutput=False)
    out_d = nc.declare_dram_parameter("out", [NH, 128, NSUB * O], bf16, isOutput=True)

    with tile.TileContext(nc) as tc:
        with (
            tc.tile_pool(name="sb", bufs=1) as sb,
            tc.tile_pool(name="sbr", bufs=2) as sbr,
            tc.tile_pool(name="sbo", bufs=1) as sbo,
            tc.tile_pool(name="pw", bufs=4, space="PSUM") as pw,
            tc.tile_pool(name="pacc", bufs=1, space="PSUM") as pacc,
        ):
            # ---- static SBUF tensors ----
            hT_sb = sb.tile([128, 2, N], bf16, name="hT_sb")
            adjT_sb = sb.tile([128, NCJ, NI], bf16, name="adjT_sb")
            wTr_sb = sb.tile([O, NH, F], bf16, name="wTr_sb")
            avec_sb = sb.tile([O, 2 * NH], bf16, name="avec_sb")
            wall_sb = sb.tile([128, 2, NH * O + NH], bf16, name="wall_sb")
            vsrc_sb = sb.tile([128, 2, NH], bf16, name="vsrc_sb")
            negrow_sb = sb.tile([1, NH, NI], bf16, name="negrow_sb")
            dneg_rep = sb.tile([128, NH, NI], bf16, name="dneg_rep")
            g_sb = sb.tile([128, NH, NCJ, O + 1], bf16, name="g_sb")
            edst3_sb = sb.tile([128, NCJ, NH], f32, name="edst3_sb")
            edst2_sb = sb.tile([128, NCJ, NH], f32, name="edst2_sb")
            ostage = sb.tile([128, NH, NSUB, O], bf16, name="ostage")
            warm_sb = sb.tile([128, 256], bf16, name="warm_sb")
            negcb = sb.tile([128, 1], f32, name="negcb")
            zerob = sb.tile([128, 1], f32, name="zerob")
            ident = sb.tile([128, 128], f32, name="ident")

            # ---- DMA in ----  DMAs drain in ISSUE ORDER at ~180 GB/s
            # effective (8 cores share HBM), so the order below is a pacing
            # schedule matched to consumption: src-chain inputs first, then
            # hT / adjT pieces interleaved in the order the chunk loop needs
            # them. hT's j-axis is host-rotated so cols [0:1024] ARE this
            # core's i-block (no separate hTi tensor).
            wTr_r = wTr_d[:, :].rearrange("o (h f) -> o h f", h=NH)
            for h in range(NH):
                nc.sync.dma_start(wTr_sb[:, h, :], wTr_r[:, h, :])
            nc.sync.dma_start(avec_sb, avec_d[:, :])
            hT_r = hT_d[:, :].rearrange("(fc p) j -> p fc j", p=128)
            nc.sync.dma_start(hT_sb[:, :, 0:512], hT_r[:, :, 0:512])
            nc.sync.dma_start(hT_sb[:, :, 512:1024], hT_r[:, :, 512:1024])
            nc.sync.dma_start(
                wall_sb[:, :, 0 : NH * O],
                wr_d[:, :].rearrange("(fc p) m -> p fc m", p=128),
            )
            adjT_r = adjT_d[:, :].rearrange("(g p) x -> g p x", p=128)
            adjT_rc = adjT_d[:, :].rearrange("(g p) (c i) -> g p c i", p=128, c=4)
            for cc in range(4):
                nc.sync.dma_start(adjT_sb[:, cc, :], adjT_rc[0, :, cc, :])
            nc.sync.dma_start(hT_sb[:, :, 1024:1536], hT_r[:, :, 1024:1536])
            for cc in range(4):
                nc.sync.dma_start(adjT_sb[:, 4 + cc, :], adjT_rc[1, :, cc, :])
            nc.sync.dma_start(hT_sb[:, :, 1536:2048], hT_r[:, :, 1536:2048])
            adjT_rh = adjT_d[:, :].rearrange("(g p) (hh x) -> g p hh x", p=128, hh=2)
            for hh in range(2):
                nc.sync.dma_start(
                    adjT_sb[:, 8 + 2 * hh : 10 + 2 * hh, :].rearrange(
                        "p c i -> p (c i)"
                    ),
                    adjT_rh[2, :, hh, :],
                )
            nc.sync.dma_start(hT_sb[:, :, 2048:3072], hT_r[:, :, 2048:3072])
            for hh in range(2):
                nc.sync.dma_start(
                    adjT_sb[:, 12 + 2 * hh : 14 + 2 * hh, :].rearrange(
                        "p c i -> p (c i)"
                    ),
                    adjT_rh[3, :, hh, :],
                )
            nc.sync.dma_start(hT_sb[:, :, 3072:4096], hT_r[:, :, 3072:4096])
            for hh in range(2):
                nc.sync.dma_start(
                    adjT_sb[:, 16 + 2 * hh : 18 + 2 * hh, :].rearrange(
                        "p c i -> p (c i)"
                    ),
                    adjT_rh[4, :, hh, :],
                )
            for g in range(5, 8):
                nc.sync.dma_start(
                    adjT_sb[:, 4 * g : 4 * g + 4, :].rearrange("p c i -> p (c i)"),
                    adjT_r[g],
                )

            # PE warm-up matmuls are interleaved between the prologue matmul
            # stages below (not all up front) so they fill the PE's DMA-wait
            # gaps without delaying the critical chain.
            nc.vector.memset(warm_sb[:, :], 1.0)

            def warm(n):
                for _ in range(n):
                    w_ps = pw.tile(
                        [128, 512], f32, name=f"warm_{warm.i}", tag="scratch"
                    )
                    warm.i += 1
                    nc.tensor.matmul(
                        w_ps[:, 0:256],
                        lhsT=warm_sb[:, 0:128],
                        rhs=warm_sb[:, :],
                        start=True,
                        stop=True,
                    )

            warm.i = 0
            warm(4)

            nc.vector.memset(negcb[:, :], -CB)
            nc.vector.memset(zerob[:, :], 0.0)
            make_identity(nc, ident[:, :])

            # ---- v vectors: v[f] = sum_o wT[o,f] * a[o]  (cols per head:
            # src, dst). One matmul per fc covers both heads' 4 columns.
            for fc in range(2):
                v_ps = pw.tile([128, 512], f32, name=f"v_ps_{fc}", tag="scratch")
                for h in range(NH):
                    nc.tensor.matmul(
                        v_ps[:, 2 * h : 2 * h + 2],
                        lhsT=wTr_sb[:, h, ts(fc, 128)],
                        rhs=avec_sb[:, 2 * h : 2 * h + 2],
                        start=True,
                        stop=True,
                    )
                for h in range(NH):
                    nc.scalar.copy(vsrc_sb[:, fc, h : h + 1], v_ps[:, 2 * h : 2 * h + 1 + 2 * h][:, 0:1])
                    nc.vector.tensor_copy(
                        wall_sb[:, fc, NH * O + h : NH * O + h + 1],
                        v_ps[:, 2 * h + 1 : 2 * h + 2],
                    )
            warm(3)

            # ---- src rows for this core's i-block: e^{-0.8 src_i}, then
            # replicate across all 128 partitions (DMA partition broadcast)
            for h in range(NH):
                for seg in range(2):
                    sr_ps = pw.tile([128, 512], f32, name=f"sr_ps_{h}_{seg}", tag="scratch")
                    for fc in range(2):
                        nc.tensor.matmul(
                            sr_ps[0:1, 0:512],
                            lhsT=vsrc_sb[:, fc, h : h + 1],
                            rhs=hT_sb[:, fc, ts(seg, 512)],
                            start=(fc == 0),
                            stop=(fc == 1),
                        )
                    nc.scalar.activation(
                        negrow_sb[0:1, h, ts(seg, 512)],
                        sr_ps[0:1, 0:512],
                        Act.Exp,
                        scale=-0.8,
                        bias=zerob[0:1, :],
                    )
                    nc.gpsimd.partition_broadcast(
                        dneg_rep[:, h, ts(seg, 512)],
                        negrow_sb[0:1, h, ts(seg, 512)],
                    )
                warm(3)

            # ---- bmm accumulators: psum [65, 512] per (head, i-segment)
            acc = [
                pacc.tile([O + 1, 512], f32, name=f"acc{g}", tag=f"acc{g}")
                for g in range(NH * NSEG)
            ]

            # ---- h_prime producer: 2 accumulated matmuls -> [h'0|h'1|dst0|dst1],
            # then ACT turns the dst columns into the two exp scalars and
            # builds G = e^{0.2 dst - CB} * [h_prime | 1] per head.
            def hp_block(c):
                hp_ps = pw.tile([128, 512], f32, name=f"hp_ps_{c}", tag="scratch")[
                    :, 0 : NH * O + NH
                ]
                for fc in range(2):
                    nc.tensor.matmul(
                        hp_ps[:, :],
                        lhsT=hT_sb[:, fc, ts(c, 128)],
                        rhs=wall_sb[:, fc, :],
                        start=(fc == 0),
                        stop=(fc == 1),
                    )
                # e^{0.8 dst} scalars for the DVE max (both heads, FD=2)
                nc.scalar.activation(
                    edst3_sb[:, c, :],
                    hp_ps[:, NH * O : NH * O + NH],
                    Act.Exp,
                    scale=0.8,
                    bias=zerob[:, :],
                )
                nc.scalar.activation(
                    edst2_sb[:, c, :],
                    hp_ps[:, NH * O : NH * O + NH],
                    Act.Exp,
                    scale=0.2,
                    bias=negcb[:, :],
                )
                for h in range(NH):
                    # G = e^{0.2 dst - CB} * [h_prime | 1]: the ones-column is
                    # the scale itself, making the bmm's last row the softmax
                    # denominator
                    nc.scalar.activation(
                        g_sb[:, h, c, 0:O],
                        hp_ps[:, ts(h, O)],
                        Act.Copy,
                        scale=edst2_sb[:, c, h : h + 1],
                    )
                    nc.scalar.copy(
                        g_sb[:, h, c, O : O + 1], edst2_sb[:, c, h : h + 1]
                    )

            for c in range(PIPE):
                hp_block(c)

            # ---- main loop, heads interleaved per chunk so the adjT demand
            # rate stays at ~125 GB/s (phasing would double it and starve on
            # DMA). Per (chunk, head): 4x-mode tensor_scalar max + 2x-mode
            # tensor_tensor adj-mask multiply on the DVE, then two 512-wide
            # bmm matmuls.
            def dve_tile(c, h):
                p_t = sbr.tile(
                    [128, NI], bf16, name=f"p_{h}_{c}", tag=f"P{h}", bufs=3
                )
                r_t = sbr.tile(
                    [128, NI], bf16, name=f"r_{h}_{c}", tag=f"R{h}", bufs=3
                )
                nc.vector.tensor_scalar(
                    out=r_t[:, :],
                    in0=dneg_rep[:, h, :],
                    scalar1=edst3_sb[:, c, h : h + 1],
                    scalar2=None,
                    op0=Alu.max,
                )
                nc.vector.tensor_tensor(
                    out=p_t[:, :],
                    in0=r_t[:, :],
                    in1=adjT_sb[:, c, :],
                    op=Alu.mult,
                )
                for seg in range(NSEG):
                    nc.tensor.matmul(
                        acc[h * NSEG + seg][:, :],
                        lhsT=g_sb[:, h, c, :],
                        rhs=p_t[:, ts(seg, 512)],
                        start=(c == 0),
                        stop=(c == NCJ - 1),
                    )

            def epilogue_seg(h, seg):
                tr_in = sbo.tile(
                    [O + 1, 512], f32, name=f"tr_{h}_{seg}", tag=f"trin{h}{seg}"
                )
                if (h + seg) % 2 == 0:
                    nc.scalar.copy(tr_in[:, :], acc[h * NSEG + seg][:, :])
                else:
                    nc.vector.tensor_copy(tr_in[:, :], acc[h * NSEG + seg][:, :])
                for q in range(4):
                    isub = seg * 4 + q
                    tr_ps = pw.tile(
                        [128, 512], f32, name=f"trp_{h}_{isub}", tag="scratch"
                    )
                    nc.tensor.transpose(
                        tr_ps[:, 0 : O + 1],
                        tr_in[:, ts(q, 128)],
                        ident[0 : O + 1, 0 : O + 1],
                    )
                    rec = sbr.tile(
                        [128, 1], f32, name=f"rec_{h}_{isub}", tag="rec", bufs=4
                    )
                    nc.vector.reciprocal(rec[:, :], tr_ps[:, O : O + 1])
                    if isub % 2 == 0:
                        nc.scalar.activation(
                            ostage[:, h, isub, :],
                            tr_ps[:, 0:O],
                            Act.Copy,
                            scale=rec[:, :],
                        )
                    else:
                        nc.vector.tensor_scalar(
                            out=ostage[:, h, isub, :],
                            in0=tr_ps[:, 0:O],
                            scalar1=rec[:, :],
                            scalar2=None,
                            op0=Alu.mult,
                        )

            for c in range(NCJ):
                if c + PIPE < NCJ:
                    hp_block(c + PIPE)
                for h in range(NH):
                    dve_tile(c, h)
            for h in range(NH):
                for seg in range(NSEG):
                    epilogue_seg(h, seg)
                nc.sync.dma_start(
                    out_d[h, :, :], ostage[:, h, :, :].rearrange("p s o -> p (s o)")
                )

    nc.finalize()
    return nc


def _prep_inputs(h, adj, w, a_src, a_dst, bias):
    """Host-side sharding / layout prep (no reference math)."""
    h = np.asarray(h, dtype=np.float32)
    adj = np.asarray(adj)
    w = np.asarray(w, dtype=np.float32)
    a_src = np.asarray(a_src, dtype=np.float32)
    a_dst = np.asarray(a_dst, dtype=np.float32)

    hT = np.ascontiguousarray(h.T)                       # [F, N]
    adjT = np.ascontiguousarray(adj.T).astype(ml_dtypes.bfloat16)  # [N, N] 0/1

    in_maps = []
    for c in range(NCORES):
        hb, ib = c % 2, c // 2
        heads = [2 * hb, 2 * hb + 1]
        i0 = NI * ib
        w2 = w[heads]                                    # [2, F, O]
        wr = np.ascontiguousarray(w2.transpose(1, 0, 2).reshape(F, NH * O))
        wTr = np.ascontiguousarray(
            np.concatenate([w2[0].T, w2[1].T], axis=1)   # [O, 2F]
        )
        avec = np.ascontiguousarray(
            np.stack(
                [a_src[heads[0], :, 0], a_dst[heads[0], :, 0],
                 a_src[heads[1], :, 0], a_dst[heads[1], :, 0]],
                axis=1,
            )
        )                                                # [O, 4]
        # roll the j axis so this core's own i-block is cols [0:1024] of
        # hT (it doubles as the src-row input) and adjT rows follow the same
        # order (the bmm sums over j, so any consistent j-order is valid)
        hT_roll = np.roll(hT, -i0, axis=1)
        adjT_roll = np.roll(adjT, -i0, axis=0)
        in_maps.append(
            {
                "hT": np.ascontiguousarray(hT_roll).astype(ml_dtypes.bfloat16),
                "adjT": np.ascontiguousarray(
                    adjT_roll[:, i0 : i0 + NI]
                    .reshape(8, 4, 128, NI)
                    .transpose(0, 2, 1, 3)
                    .reshape(8 * 128, 4 * NI)
                ),
                "wr": wr.astype(ml_dtypes.bfloat16),
                "wTr": wTr.astype(ml_dtypes.bfloat16),
                "avec": avec.astype(ml_dtypes.bfloat16),
            }
        )
    return in_maps


def kernel(h, adj, w, a_src, a_dst, bias):
    from concourse.bass_utils import run_bass_kernel_spmd

    if "nc" not in _CACHE:
        _CACHE["nc"] = _build()
    nc = _CACHE["nc"]

    in_maps = _prep_inputs(h, adj, w, a_src, a_dst, bias)
    res = run_bass_kernel_spmd(nc, in_maps, list(range(NCORES))).results

    out = np.empty((N, NHEAD, O), dtype=np.float32)
    for c in range(NCORES):
        hb, ib = c % 2, c // 2
        arr = np.asarray(res[c]["out"], dtype=np.float32)  # [NH, 128, NSUB*O]
        for hh in range(NH):
            blk = (
                arr[hh]
                .reshape(128, NSUB, O)
                .transpose(1, 0, 2)
                .reshape(NI, O)
            )
            out[NI * ib : NI * (ib + 1), 2 * hb + hh, :] = blk
    out += np.asarray(bias, dtype=np.float32).reshape(1, 1, O)
    return out
